# revision 17
# baseline (speedup 1.0000x reference)
"""Trainium2 Bass kernel for nn_LogicLayer (differentiable logic-gate layer).

Reference computation:
    a = x[:, idx_a]; b = x[:, idx_b]                  # [B, OUT] gathers
    w = softmax(weights, -1)                          # [OUT, 16]
    out = sum_k w[:, k] * gate_k(a, b)

Every gate value is of the form c0 + c1*a + c2*b + c3*a*b, so
    out[i, j] = W0[j] + W1[j]*a + W2[j]*b + W3[j]*a*b
with W = softmax(weights) @ C, C the [16, 4] gate-coefficient table.

Kernel strategy (out_dim-parallel across 8 cores, 1024 neurons/core):
  host: W coefficients (softmax @ C, tiny), x transposed+cast to fp16
        xT16 [IN, B] passed as the gather table, per-core idx packing.
  device (per core, its 1024 j's, full batch on the free axis):
    1. dma_gather rows xT16[idx[j], :] -- idx_a and idx_b CONCATENATED per
       chunk so one SWDGE call fetches both operands (5 calls total, under
       the 8-deep SWDGE sem pool; 4 KiB per gathered row)
    2. s = W3*b + W1 (ACT), q = W2*b + W0 (DVE ts, 4x fp16 mode),
       m = a*s, o = m + q as multi-slot DVE tensor_tensor ops
    3. store o to outT [1024, B] fp16 (4 KiB partition lines)
  host: assemble outT -> transpose -> float32 full output.

No PE/PSUM use at all and ~12 MiB HBM traffic per core vs ~41 MiB for
the batch-parallel transpose-on-device variant.
"""

import numpy as np

# ---------------------------------------------------------------- constants
B_TOT, IN_DIM, OUT_DIM = 2048, 8192, 8192
NCORES = 8
NJ_CORE = OUT_DIM // NCORES     # 1024 output neurons per core
# j-slots (128 each) per gather chunk; a+b idxs combined in one call
CHUNK_SLOTS = [1, 2, 2, 2, 1]
assert sum(CHUNK_SLOTS) * 128 == NJ_CORE

# value = c0 + c1*a + c2*b + c3*ab  for each of the 16 gates
GATE_C = np.array(
    [
        # c0  c1  c2  c3
        [0, 0, 0, 0],    # 0  False
        [0, 0, 0, 1],    # 1  a AND b
        [0, 1, 0, -1],   # 2  a AND NOT b
        [0, 1, 0, 0],    # 3  a
        [0, 0, 1, -1],   # 4  NOT a AND b
        [0, 0, 1, 0],    # 5  b
        [0, 1, 1, -2],   # 6  a XOR b
        [0, 1, 1, -1],   # 7  a OR b
        [1, -1, -1, 1],  # 8  NOT (a OR b)
        [1, -1, -1, 2],  # 9  NOT (a XOR b)
        [1, 0, -1, 0],   # 10 NOT b
        [1, 0, -1, 1],   # 11 a OR NOT b
        [1, -1, 0, 0],   # 12 NOT a
        [1, -1, 0, 1],   # 13 NOT a OR b
        [1, 0, 0, -1],   # 14 NOT (a AND b)
        [1, 0, 0, 0],    # 15 True
    ],
    dtype=np.float64,
)  # [16, 4]


# ---------------------------------------------------------------- device IR
def build_nc(NJ=NJ_CORE, IN=IN_DIM, B=B_TOT):
    """Build the per-core Bass module (SPMD; all cores run the same IR)."""
    import sys

    if "/opt/trn_rl_repo" not in sys.path:
        sys.path.insert(0, "/opt/trn_rl_repo")

    import concourse.tile as tile
    from concourse import bacc, mybir, library_config
    from contextlib import ExitStack

    f32 = mybir.dt.float32
    f16 = mybir.dt.float16
    i16 = mybir.dt.int16
    SLOTS = NJ // 128          # 8 j-slots per core

    nc = bacc.Bacc("TRN2", target_bir_lowering=False)
    xt = nc.declare_dram_parameter("xt16", [IN, B], f16, isOutput=False)
    wc = nc.declare_dram_parameter("wcoef", [128, 4 * SLOTS], f32, isOutput=False)
    ix = nc.declare_dram_parameter("idx16", [128, 2 * NJ // 16], i16, isOutput=False)
    outt = nc.declare_dram_parameter("outt", [NJ, B], f16, isOutput=True)

    Ident = mybir.ActivationFunctionType.Identity
    MULT = mybir.AluOpType.mult
    ADD = mybir.AluOpType.add

    with tile.TileContext(nc) as tc, ExitStack() as ctx:
        # kick the Q7 gather-lib swap off as early as possible: its ~9us
        # load latency gates the first dma_gather desc-gen
        nc.gpsimd.load_library(library_config.mlp)

        cpool = ctx.enter_context(tc.tile_pool(name="consts", bufs=1))
        ixt = cpool.tile([128, 2 * NJ // 16], i16, name="ixt")
        nc.sync.dma_start(ixt[:], ix[:])
        wct = cpool.tile([128, 4 * SLOTS], f32, name="wct")
        nc.sync.dma_start(wct[:], wc[:])

        # one MOVE per distinct gather size instead of one per call
        sizes = sorted({2 * s * 128 for s in CHUNK_SLOTS})
        nregs = {n: nc.gpsimd.to_reg(n) for n in sizes}

        gpool = ctx.enter_context(tc.tile_pool(name="gath", bufs=1))
        spool = ctx.enter_context(tc.tile_pool(name="sqm", bufs=2))
        opool = ctx.enter_context(tc.tile_pool(name="out", bufs=2))

        def wap(k, c):  # [128, 1] f32 per-partition scalar for W_k, slot c
            return wct[:, k * SLOTS + c:k * SLOTS + c + 1]

        nch = len(CHUNK_SLOTS)
        c0 = 0
        icol = 0
        for ci, sl_n in enumerate(CHUNK_SLOTS):
            nidx = 2 * sl_n * 128
            # combined gather: slots [0, sl_n) = a rows, [sl_n, 2*sl_n) = b
            gab = gpool.tile([128, 2 * sl_n, B], f16, tag=f"g{ci}")
            nc.gpsimd.dma_gather(
                gab[:], xt[:], ixt[:, icol:icol + nidx // 16],
                nidx, nregs[nidx], B
            )
            icol += nidx // 16
            tail = ci == nch - 1

            if not tail:
                s2 = spool.tile([128, sl_n, B], f16, tag="s")
                q2 = spool.tile([128, sl_n, B], f16, tag="q")
                for sl in range(sl_n):
                    c = c0 + sl
                    nc.scalar.activation(
                        s2[:, sl, :], gab[:, sl_n + sl, :], Ident,
                        scale=wap(3, c), bias=wap(1, c),
                    )
                    nc.vector.tensor_scalar(
                        q2[:, sl, :], gab[:, sl_n + sl, :],
                        wap(2, c), wap(0, c), op0=MULT, op1=ADD,
                    )
                m2 = spool.tile([128, sl_n, B], f16, tag="m")
                nc.vector.tensor_tensor(
                    m2[:], gab[:, 0:sl_n, :], s2[:], op=MULT
                )
                o2 = opool.tile([128, sl_n, B], f16, tag="o")
                nc.vector.tensor_tensor(o2[:], m2[:], q2[:], op=ADD)
                nc.sync.dma_start(
                    outt[c0 * 128:(c0 + sl_n) * 128, :]
                    .rearrange("(s p) b -> p s b", p=128),
                    o2[:],
                )
            else:
                # tail: all on DVE (cheap ts there), batch-split halves to
                # shorten the un-overlapped chain after the last gather
                c = c0
                for hi, hs in enumerate((slice(0, B // 2), slice(B // 2, B))):
                    hb = hs.stop - hs.start
                    st = spool.tile([128, 1, B], f16, tag="s")
                    nc.vector.tensor_scalar(
                        st[:, 0, :hb], gab[:, 1, hs],
                        wap(3, c), wap(1, c), op0=MULT, op1=ADD,
                    )
                    qt = spool.tile([128, 1, B], f16, tag="q")
                    nc.vector.tensor_scalar(
                        qt[:, 0, :hb], gab[:, 1, hs],
                        wap(2, c), wap(0, c), op0=MULT, op1=ADD,
                    )
                    mt = spool.tile([128, 1, B], f16, tag="m")
                    nc.vector.tensor_tensor(
                        mt[:, 0, :hb], gab[:, 0, hs], st[:, 0, :hb], op=MULT
                    )
                    ot = opool.tile([128, B], f16, tag=f"ot{hi}")
                    nc.vector.tensor_tensor(
                        ot[:, :hb], mt[:, 0, :hb], qt[:, 0, :hb], op=ADD
                    )
                    nc.sync.dma_start(
                        outt[c * 128:(c + 1) * 128, hs], ot[:, :hb]
                    )
            c0 += sl_n
    nc.compile()
    return nc


# ---------------------------------------------------------------- host side
def _wrap_block(idx):
    """Pack one chunk's index list into dma_gather's wrapped int16 layout:
    idx16[p, s] = idx[s*16 + p%16], replicated over 8 groups of 16
    partitions. Returns [128, len(idx)//16]."""
    n = len(idx)
    a = np.asarray(idx).astype(np.int16).reshape(n // 16, 16)  # [s, p]
    return np.tile(a.T, (8, 1))                                # [128, n//16]


def _pack_idx(idx_a, idx_b):
    """Per chunk, concatenate the a-idxs and b-idxs so a single dma_gather
    fetches both operands."""
    blocks = []
    lo = 0
    for sl_n in CHUNK_SLOTS:
        hi = lo + sl_n * 128
        blocks.append(_wrap_block(np.concatenate([idx_a[lo:hi], idx_b[lo:hi]])))
        lo = hi
    return np.ascontiguousarray(np.concatenate(blocks, axis=1))


def _prep_inputs(x, weights, idx_a, idx_b):
    x = np.asarray(x, dtype=np.float32)
    w = np.asarray(weights, dtype=np.float64)
    e = np.exp(w - w.max(axis=-1, keepdims=True))
    sm = e / e.sum(axis=-1, keepdims=True)
    W4 = (sm @ GATE_C).astype(np.float32)                      # [OUT, 4]

    xt16 = x.T.astype(np.float16, order="C")                   # [IN, B]
    idx_a = np.asarray(idx_a)
    idx_b = np.asarray(idx_b)

    SLOTS = NJ_CORE // 128
    in_maps = []
    for c in range(NCORES):
        j0 = c * NJ_CORE
        # wcoef[q, k*SLOTS + c] = W4[j0 + c*128 + q, k]
        wcoef = np.ascontiguousarray(
            W4[j0:j0 + NJ_CORE]
            .reshape(SLOTS, 128, 4)
            .transpose(1, 2, 0)
            .reshape(128, 4 * SLOTS)
        )
        in_maps.append(
            {
                "xt16": xt16,
                "wcoef": wcoef,
                "idx16": _pack_idx(idx_a[j0:j0 + NJ_CORE],
                                   idx_b[j0:j0 + NJ_CORE]),
            }
        )
    return in_maps


_NC_CACHE = {}


def _get_nc():
    if "nc" not in _NC_CACHE:
        _NC_CACHE["nc"] = build_nc()
    return _NC_CACHE["nc"]


def _post(res, inputs=None):
    outt = np.concatenate([r["outt"] for r in res.results], axis=0)  # [OUT, B]
    return outt.T.astype(np.float32, order="C")


def kernel(x, weights, idx_a, idx_b):
    import sys

    if "/opt/trn_rl_repo" not in sys.path:
        sys.path.insert(0, "/opt/trn_rl_repo")
    from concourse.bass_utils import run_bass_kernel_spmd

    nc = _get_nc()
    in_maps = _prep_inputs(x, weights, idx_a, idx_b)
    res = run_bass_kernel_spmd(nc, in_maps, list(range(NCORES)))
    return _post(res)


if __name__ == "__main__":
    nc = build_nc()
    print("built OK")


# revision 19
# speedup vs baseline: 1.1621x; 1.1621x over previous
"""Trainium2 Bass kernel for nn_LogicLayer (differentiable logic-gate layer).

Reference computation:
    a = x[:, idx_a]; b = x[:, idx_b]                  # [B, OUT] gathers
    w = softmax(weights, -1)                          # [OUT, 16]
    out = sum_k w[:, k] * gate_k(a, b)

Every gate value is of the form c0 + c1*a + c2*b + c3*a*b, so
    out[i, j] = W0[j] + W1[j]*a + W2[j]*b + W3[j]*a*b
with W = softmax(weights) @ C, C the [16, 4] gate-coefficient table.

Kernel strategy (out_dim-parallel across 8 cores, 1024 neurons/core):
  host: W coefficients (softmax @ C, tiny), x transposed+cast to fp16
        xT16 [IN, B] passed as the gather table, per-core idx packing.
  device (per core, its 1024 j's = 8 slots of 128, batch on free axis):
    1. per slot, ONE dma_gather with idx_a||idx_b concatenated (256 rows,
       4 KiB each): 8 SWDGE calls total -- stays under the 8-deep SWDGE
       sem pool (more calls stall ~8us on sem reuse)
    2. s = W3*b + W1 (ACT), q = W2*b + W0 (DVE ts, 4x fp16), m = a*s,
       o = m + q (DVE tt, strictly 2D [128, B] APs -- 3D APs drop the
       DVE 2x fp16 mode). Two mid-slot q's on ACT to balance engines.
       Final slot entirely on DVE, batch-split, to shorten the tail chain.
    3. store o to outT [1024, B] fp16 (4 KiB partition lines)
  host: assemble outT -> transpose -> float32 full output.

No PE/PSUM use; ~12 MiB HBM traffic per core vs ~41 MiB for the
batch-parallel transpose-on-device variant.
"""

import numpy as np

# ---------------------------------------------------------------- constants
B_TOT, IN_DIM, OUT_DIM = 2048, 8192, 8192
NCORES = 8
NJ_CORE = OUT_DIM // NCORES     # 1024 output neurons per core

# value = c0 + c1*a + c2*b + c3*ab  for each of the 16 gates
GATE_C = np.array(
    [
        # c0  c1  c2  c3
        [0, 0, 0, 0],    # 0  False
        [0, 0, 0, 1],    # 1  a AND b
        [0, 1, 0, -1],   # 2  a AND NOT b
        [0, 1, 0, 0],    # 3  a
        [0, 0, 1, -1],   # 4  NOT a AND b
        [0, 0, 1, 0],    # 5  b
        [0, 1, 1, -2],   # 6  a XOR b
        [0, 1, 1, -1],   # 7  a OR b
        [1, -1, -1, 1],  # 8  NOT (a OR b)
        [1, -1, -1, 2],  # 9  NOT (a XOR b)
        [1, 0, -1, 0],   # 10 NOT b
        [1, 0, -1, 1],   # 11 a OR NOT b
        [1, -1, 0, 0],   # 12 NOT a
        [1, -1, 0, 1],   # 13 NOT a OR b
        [1, 0, 0, -1],   # 14 NOT (a AND b)
        [1, 0, 0, 0],    # 15 True
    ],
    dtype=np.float64,
)  # [16, 4]


# ---------------------------------------------------------------- device IR
def build_nc(NJ=NJ_CORE, IN=IN_DIM, B=B_TOT):
    """Build the per-core Bass module (SPMD; all cores run the same IR)."""
    import sys

    if "/opt/trn_rl_repo" not in sys.path:
        sys.path.insert(0, "/opt/trn_rl_repo")

    import concourse.tile as tile
    from concourse import bacc, mybir, library_config
    from contextlib import ExitStack

    f32 = mybir.dt.float32
    f16 = mybir.dt.float16
    i16 = mybir.dt.int16
    SLOTS = NJ // 128          # 8 j-slots per core

    nc = bacc.Bacc("TRN2", target_bir_lowering=False)
    xt = nc.declare_dram_parameter("xt16", [IN, B], f16, isOutput=False)
    wc = nc.declare_dram_parameter("wcoef", [128, 4 * SLOTS], f32, isOutput=False)
    ix = nc.declare_dram_parameter("idx16", [128, 2 * NJ // 16], i16, isOutput=False)
    outt = nc.declare_dram_parameter("outt", [NJ, B], f16, isOutput=True)

    Ident = mybir.ActivationFunctionType.Identity
    MULT = mybir.AluOpType.mult
    ADD = mybir.AluOpType.add

    with tile.TileContext(nc) as tc, ExitStack() as ctx:
        # kick the Q7 gather-lib swap off as early as possible: its ~9us
        # load latency gates the first dma_gather desc-gen
        nc.gpsimd.load_library(library_config.mlp)

        cpool = ctx.enter_context(tc.tile_pool(name="consts", bufs=1))
        ixt = cpool.tile([128, 2 * NJ // 16], i16, name="ixt")
        nc.sync.dma_start(ixt[:], ix[:])
        wct = cpool.tile([128, 4 * SLOTS], f32, name="wct")
        nc.sync.dma_start(wct[:], wc[:])

        nreg = nc.gpsimd.to_reg(256)

        gpool = ctx.enter_context(tc.tile_pool(name="gath", bufs=1))
        spool = ctx.enter_context(tc.tile_pool(name="sqm", bufs=3))
        opool = ctx.enter_context(tc.tile_pool(name="out", bufs=4))

        def wap(k, c):  # [128, 1] f32 per-partition scalar for W_k, slot c
            return wct[:, k * SLOTS + c:k * SLOTS + c + 1]

        ACT_Q_SLOTS = {4, 5}   # mid-slot q's on ACT: it has slack there

        for c in range(SLOTS):
            # combined gather: 128 a-rows (sub-slot 0) + 128 b-rows (1)
            gab = gpool.tile([128, 2, B], f16, tag=f"g{c}")
            nc.gpsimd.dma_gather(
                gab[:], xt[:], ixt[:, c * 16:(c + 1) * 16], 256, nreg, B
            )
            ga, gb = gab[:, 0, :], gab[:, 1, :]
            if c < SLOTS - 1:
                s = spool.tile([128, B], f16, tag="s")
                nc.scalar.activation(
                    s[:], gb, Ident, scale=wap(3, c), bias=wap(1, c)
                )
                q = spool.tile([128, B], f16, tag="q")
                if c in ACT_Q_SLOTS:
                    nc.scalar.activation(
                        q[:], gb, Ident, scale=wap(2, c), bias=wap(0, c)
                    )
                else:
                    nc.vector.tensor_scalar(
                        q[:], gb, wap(2, c), wap(0, c), op0=MULT, op1=ADD
                    )
                m = spool.tile([128, B], f16, tag="m")
                nc.vector.tensor_tensor(m[:], ga, s[:], op=MULT)
                o = opool.tile([128, B], f16, tag="o")
                nc.vector.tensor_tensor(o[:], m[:], q[:], op=ADD)
                nc.sync.dma_start(outt[c * 128:(c + 1) * 128, :], o[:])
            else:
                # tail slot: all ops on DVE (ts is 2.6x cheaper there than
                # ACT), split into batch halves for a short finish chain
                for hi, hs in enumerate((slice(0, B // 2), slice(B // 2, B))):
                    hb = hs.stop - hs.start
                    s = spool.tile([128, B], f16, tag="s")
                    nc.vector.tensor_scalar(
                        s[:, :hb], gb[:, hs], wap(3, c), wap(1, c),
                        op0=MULT, op1=ADD,
                    )
                    q = spool.tile([128, B], f16, tag="q")
                    nc.vector.tensor_scalar(
                        q[:, :hb], gb[:, hs], wap(2, c), wap(0, c),
                        op0=MULT, op1=ADD,
                    )
                    m = spool.tile([128, B], f16, tag="m")
                    nc.vector.tensor_tensor(
                        m[:, :hb], ga[:, hs], s[:, :hb], op=MULT
                    )
                    o = opool.tile([128, B], f16, tag=f"ot{hi}")
                    nc.vector.tensor_tensor(
                        o[:, :hb], m[:, :hb], q[:, :hb], op=ADD
                    )
                    nc.sync.dma_start(
                        outt[c * 128:(c + 1) * 128, hs], o[:, :hb]
                    )
    nc.compile()
    return nc


# ---------------------------------------------------------------- host side
def _wrap_block(idx):
    """Pack one call's index list into dma_gather's wrapped int16 layout:
    idx16[p, s] = idx[s*16 + p%16], replicated over 8 groups of 16
    partitions. Returns [128, len(idx)//16]."""
    n = len(idx)
    a = np.asarray(idx).astype(np.int16).reshape(n // 16, 16)  # [s, p]
    return np.tile(a.T, (8, 1))                                # [128, n//16]


def _pack_idx(idx_a, idx_b):
    """Per 128-j slot, concatenate the a-idxs and b-idxs so a single
    dma_gather fetches both operands."""
    blocks = []
    for lo in range(0, NJ_CORE, 128):
        blocks.append(
            _wrap_block(np.concatenate([idx_a[lo:lo + 128],
                                        idx_b[lo:lo + 128]]))
        )
    return np.ascontiguousarray(np.concatenate(blocks, axis=1))


def _prep_inputs(x, weights, idx_a, idx_b):
    x = np.asarray(x, dtype=np.float32)
    w = np.asarray(weights, dtype=np.float64)
    e = np.exp(w - w.max(axis=-1, keepdims=True))
    sm = e / e.sum(axis=-1, keepdims=True)
    W4 = (sm @ GATE_C).astype(np.float32)                      # [OUT, 4]

    xt16 = x.T.astype(np.float16, order="C")                   # [IN, B]
    idx_a = np.asarray(idx_a)
    idx_b = np.asarray(idx_b)

    SLOTS = NJ_CORE // 128
    in_maps = []
    for c in range(NCORES):
        j0 = c * NJ_CORE
        # wcoef[q, k*SLOTS + c] = W4[j0 + c*128 + q, k]
        wcoef = np.ascontiguousarray(
            W4[j0:j0 + NJ_CORE]
            .reshape(SLOTS, 128, 4)
            .transpose(1, 2, 0)
            .reshape(128, 4 * SLOTS)
        )
        in_maps.append(
            {
                "xt16": xt16,
                "wcoef": wcoef,
                "idx16": _pack_idx(idx_a[j0:j0 + NJ_CORE],
                                   idx_b[j0:j0 + NJ_CORE]),
            }
        )
    return in_maps


_NC_CACHE = {}


def _get_nc():
    if "nc" not in _NC_CACHE:
        _NC_CACHE["nc"] = build_nc()
    return _NC_CACHE["nc"]


def _post(res, inputs=None):
    outt = np.concatenate([r["outt"] for r in res.results], axis=0)  # [OUT, B]
    return outt.T.astype(np.float32, order="C")


def kernel(x, weights, idx_a, idx_b):
    import sys

    if "/opt/trn_rl_repo" not in sys.path:
        sys.path.insert(0, "/opt/trn_rl_repo")
    from concourse.bass_utils import run_bass_kernel_spmd

    nc = _get_nc()
    in_maps = _prep_inputs(x, weights, idx_a, idx_b)
    res = run_bass_kernel_spmd(nc, in_maps, list(range(NCORES)))
    return _post(res)


if __name__ == "__main__":
    nc = build_nc()
    print("built OK")


# revision 21
# speedup vs baseline: 1.2698x; 1.0926x over previous
"""Trainium2 Bass kernel for nn_LogicLayer (differentiable logic-gate layer).

Reference computation:
    a = x[:, idx_a]; b = x[:, idx_b]                  # [B, OUT] gathers
    w = softmax(weights, -1)                          # [OUT, 16]
    out = sum_k w[:, k] * gate_k(a, b)

Every gate value is of the form c0 + c1*a + c2*b + c3*a*b, so
    out[i, j] = W0[j] + W1[j]*a + W2[j]*b + W3[j]*a*b
with W = softmax(weights) @ C, C the [16, 4] gate-coefficient table.

Kernel strategy (out_dim-parallel across 8 cores, 1024 neurons/core):
  host: W coefficients (softmax @ C, tiny), x transposed+cast to fp16
        xT16 [IN, B] passed as the gather table, per-core idx packing.
  device (per core, its 1024 j's, full batch on the free axis):
    1. dma_gather rows xT16[idx_a[j], :] and xT16[idx_b[j], :]
       (j on partitions, 4 KiB per gathered row -> efficient SWDGE DMA)
    2. s = W3*b + W1 (ACT), q = W2*b + W0 (DVE ts, 4x fp16 mode),
       m = a*s (DVE tt), o = m + q (DVE tt)
    3. store o to outT [1024, B] fp16 (4 KiB partition lines)
  host: assemble outT -> transpose -> float32 full output.

No PE/PSUM use at all and ~12 MiB HBM traffic per core vs ~41 MiB for
the batch-parallel transpose-on-device variant.
"""

import numpy as np

# ---------------------------------------------------------------- constants
B_TOT, IN_DIM, OUT_DIM = 2048, 8192, 8192
NCORES = 8
NJ_CORE = OUT_DIM // NCORES     # 1024 output neurons per core
CHUNK = 256                     # idxs per dma_gather call (2 slots of 128)

# value = c0 + c1*a + c2*b + c3*ab  for each of the 16 gates
GATE_C = np.array(
    [
        # c0  c1  c2  c3
        [0, 0, 0, 0],    # 0  False
        [0, 0, 0, 1],    # 1  a AND b
        [0, 1, 0, -1],   # 2  a AND NOT b
        [0, 1, 0, 0],    # 3  a
        [0, 0, 1, -1],   # 4  NOT a AND b
        [0, 0, 1, 0],    # 5  b
        [0, 1, 1, -2],   # 6  a XOR b
        [0, 1, 1, -1],   # 7  a OR b
        [1, -1, -1, 1],  # 8  NOT (a OR b)
        [1, -1, -1, 2],  # 9  NOT (a XOR b)
        [1, 0, -1, 0],   # 10 NOT b
        [1, 0, -1, 1],   # 11 a OR NOT b
        [1, -1, 0, 0],   # 12 NOT a
        [1, -1, 0, 1],   # 13 NOT a OR b
        [1, 0, 0, -1],   # 14 NOT (a AND b)
        [1, 0, 0, 0],    # 15 True
    ],
    dtype=np.float64,
)  # [16, 4]


# ---------------------------------------------------------------- device IR
def build_nc(NJ=NJ_CORE, IN=IN_DIM, B=B_TOT):
    """Build the per-core Bass module (SPMD; all cores run the same IR)."""
    import sys

    if "/opt/trn_rl_repo" not in sys.path:
        sys.path.insert(0, "/opt/trn_rl_repo")

    import concourse.tile as tile
    from concourse import bacc, mybir, library_config
    from contextlib import ExitStack

    f32 = mybir.dt.float32
    f16 = mybir.dt.float16
    i16 = mybir.dt.int16
    SLOTS = NJ // 128          # 8 j-slots per core
    # small first chunk -> compute ramps early; small last -> short tail
    CHUNKS = [128, 256, 256, 256, 128]
    assert sum(CHUNKS) == NJ

    nc = bacc.Bacc("TRN2", target_bir_lowering=False)
    xt = nc.declare_dram_parameter("xt16", [IN, B], f16, isOutput=False)
    wc = nc.declare_dram_parameter("wcoef", [128, 4 * SLOTS], f32, isOutput=False)
    ia = nc.declare_dram_parameter("idxa16", [128, NJ // 16], i16, isOutput=False)
    ib = nc.declare_dram_parameter("idxb16", [128, NJ // 16], i16, isOutput=False)
    outt = nc.declare_dram_parameter("outt", [NJ, B], f16, isOutput=True)

    Ident = mybir.ActivationFunctionType.Identity
    MULT = mybir.AluOpType.mult
    ADD = mybir.AluOpType.add

    with tile.TileContext(nc) as tc, ExitStack() as ctx:
        # kick the Q7 gather-lib swap off as early as possible: its ~9us
        # load latency gates the first dma_gather desc-gen
        nc.gpsimd.load_library(library_config.mlp)

        cpool = ctx.enter_context(tc.tile_pool(name="consts", bufs=1))
        iat = cpool.tile([128, NJ // 16], i16, name="iat")
        nc.sync.dma_start(iat[:], ia[:])
        ibt = cpool.tile([128, NJ // 16], i16, name="ibt")
        nc.sync.dma_start(ibt[:], ib[:])
        wct = cpool.tile([128, 4 * SLOTS], f32, name="wct")
        nc.sync.dma_start(wct[:], wc[:])

        # one MOVE per distinct chunk size instead of one per gather call
        # (each MOVE costs ~0.4us of GPSIMD sequencer time up front)
        nregs = {n: nc.gpsimd.to_reg(n) for n in sorted(set(CHUNKS))}

        gpool = ctx.enter_context(tc.tile_pool(name="gath", bufs=1))
        spool = ctx.enter_context(tc.tile_pool(name="sqm", bufs=3))
        opool = ctx.enter_context(tc.tile_pool(name="out", bufs=4))
        dpool = ctx.enter_context(tc.tile_pool(name="defer", bufs=1))

        def wap(k, c):  # [128, 1] f32 per-partition scalar for W_k, slot c
            return wct[:, k * SLOTS + c:k * SLOTS + c + 1]

        ACT_Q_SLOTS = {5, 6}   # late q's on ACT: it has slack there, DVE not
        POOL_O_SLOTS = set()   # (gpsimd tensor_tensor measured 4.7us/op: too slow; Pool is
        #                        done with desc-gen by then and otherwise idle
        last_c = NJ // 128 - 1
        deferred = []

        off = 0
        for ci, n in enumerate(CHUNKS):
            sl_n = n // 128
            icol0, icol1 = off // 16, (off + n) // 16
            # b feeds both s and q -> gather it first
            gb = gpool.tile([128, sl_n, B], f16, tag=f"gb{ci}")
            nc.gpsimd.dma_gather(
                gb[:], xt[:], ibt[:, icol0:icol1], n, nregs[n], B
            )
            ga = gpool.tile([128, sl_n, B], f16, tag=f"ga{ci}")
            nc.gpsimd.dma_gather(
                ga[:], xt[:], iat[:, icol0:icol1], n, nregs[n], B
            )
            for sl in range(sl_n):
                c = off // 128 + sl
                # final slot: split by batch halves to shorten the
                # un-overlapped dependence chain after the last gather
                hsplit = [slice(0, B // 2), slice(B // 2, B)] if c == last_c \
                    else [slice(0, B)]
                for hi, hs in enumerate(hsplit):
                    hb = hs.stop - hs.start
                    s = spool.tile([128, B], f16, tag="s")
                    nc.scalar.activation(
                        s[:, :hb], gb[:, sl, hs], Ident,
                        scale=wap(3, c), bias=wap(1, c),
                    )
                    qp = dpool if c in POOL_O_SLOTS else spool
                    q = qp.tile([128, B], f16, tag=f"qd{c}" if c in POOL_O_SLOTS else "q")
                    if c in ACT_Q_SLOTS:
                        nc.scalar.activation(
                            q[:, :hb], gb[:, sl, hs], Ident,
                            scale=wap(2, c), bias=wap(0, c),
                        )
                    else:
                        nc.vector.tensor_scalar(
                            q[:, :hb], gb[:, sl, hs], wap(2, c), wap(0, c),
                            op0=MULT, op1=ADD,
                        )
                    qknown = q  # keep name for deferred capture
                    mp = dpool if c in POOL_O_SLOTS else spool
                    m = mp.tile([128, B], f16, tag=f"md{c}" if c in POOL_O_SLOTS else "m")
                    nc.vector.tensor_tensor(
                        m[:, :hb], ga[:, sl, hs], s[:, :hb], op=MULT
                    )
                    if c in POOL_O_SLOTS:
                        deferred.append((c, hs, hb, m, qknown))
                        continue
                    o = opool.tile([128, B], f16, tag=f"o{hi}")
                    nc.vector.tensor_tensor(
                        o[:, :hb], m[:, :hb], q[:, :hb], op=ADD
                    )
                    nc.sync.dma_start(
                        outt[c * 128:(c + 1) * 128, hs], o[:, :hb]
                    )
            off += n

        # deferred o-adds on GPSIMD, after every dma_gather in program order
        for c, hs, hb, m, q in deferred:
            o = dpool.tile([128, B], f16, tag=f"op{c}")
            nc.gpsimd.tensor_tensor(o[:, :hb], m[:, :hb], q[:, :hb], op=ADD)
            nc.sync.dma_start(outt[c * 128:(c + 1) * 128, hs], o[:, :hb])
    nc.compile()
    return nc


# ---------------------------------------------------------------- host side
def _wrap_idx(idx):
    """Pack an index vector into dma_gather's wrapped int16 layout:
    idx16[p, s] = idx[s*16 + p%16], replicated over the 8 groups of 16
    partitions."""
    n = len(idx)
    a = np.asarray(idx).astype(np.int16).reshape(n // 16, 16)  # [s, p]
    return np.ascontiguousarray(np.tile(a.T, (8, 1)))          # [128, n//16]


def _prep_inputs(x, weights, idx_a, idx_b):
    x = np.asarray(x, dtype=np.float32)
    w = np.asarray(weights, dtype=np.float64)
    e = np.exp(w - w.max(axis=-1, keepdims=True))
    sm = e / e.sum(axis=-1, keepdims=True)
    W4 = (sm @ GATE_C).astype(np.float32)                      # [OUT, 4]

    xt16 = x.T.astype(np.float16, order="C")                   # [IN, B]
    idx_a = np.asarray(idx_a)
    idx_b = np.asarray(idx_b)

    SLOTS = NJ_CORE // 128
    in_maps = []
    for c in range(NCORES):
        j0 = c * NJ_CORE
        # wcoef[q, k*SLOTS + c] = W4[j0 + c*128 + q, k]
        wcoef = np.ascontiguousarray(
            W4[j0:j0 + NJ_CORE]
            .reshape(SLOTS, 128, 4)
            .transpose(1, 2, 0)
            .reshape(128, 4 * SLOTS)
        )
        in_maps.append(
            {
                "xt16": xt16,
                "wcoef": wcoef,
                "idxa16": _wrap_idx(idx_a[j0:j0 + NJ_CORE]),
                "idxb16": _wrap_idx(idx_b[j0:j0 + NJ_CORE]),
            }
        )
    return in_maps


_NC_CACHE = {}


def _get_nc():
    if "nc" not in _NC_CACHE:
        _NC_CACHE["nc"] = build_nc()
    return _NC_CACHE["nc"]


def _post(res, inputs=None):
    outt = np.concatenate([r["outt"] for r in res.results], axis=0)  # [OUT, B]
    return outt.T.astype(np.float32, order="C")


def kernel(x, weights, idx_a, idx_b):
    import sys

    if "/opt/trn_rl_repo" not in sys.path:
        sys.path.insert(0, "/opt/trn_rl_repo")
    from concourse.bass_utils import run_bass_kernel_spmd

    nc = _get_nc()
    in_maps = _prep_inputs(x, weights, idx_a, idx_b)
    res = run_bass_kernel_spmd(nc, in_maps, list(range(NCORES)))
    return _post(res)


if __name__ == "__main__":
    nc = build_nc()
    print("built OK")
